# revision 6
# baseline (speedup 1.0000x reference)
"""Attention pooling kernel for Trainium2 (8 NeuronCores, SPMD batch-parallel).

Math (per batch row b):
    scores = h[b] @ query / sqrt(H)          # [L]
    weights = softmax(scores + mask_term)    # [L]
    out[b] = weights @ h[b]                  # [H]

Strategy (per core, 4 rows of B=32):
  - Stream h[b] in 1 MiB DMA pairs ([128, 2, 1024] tiles, partition = L%128).
  - DVE affine_mul_reduce computes the per-chunk dot products with a
    broadcast query tile in one pass (accum_out = per-partition sum).
  - ScalarE exp over groups of 4 chunks (scale folds the 1/sqrt(H)), with
    accum_out accumulating the softmax normalizer Z.
  - PE accumulates out = sum_l w~[l] * h[l,:] into PSUM [1, 1024] via
    M=1 matmuls (lhsT = weight column, rhs = h tile), plus a final
    ones-matmul to reduce Z across partitions.
  - Scores are tiny (|s| < ~0.2) so softmax needs no max subtraction;
    exp() is exact to ~2 ULP on that range and Z accumulates unshifted.
  - h is read from HBM exactly once: ~64 MiB/core => DMA-roofline bound.
"""

import sys

if "/opt/trn_rl_repo" not in sys.path:
    sys.path.insert(0, "/opt/trn_rl_repo")

import json

import numpy as np

B, L, H = 32, 4096, 1024
N_CORES = 8
B_LOCAL = B // N_CORES  # 4
P = 128
NCHUNK = L // P  # 32
PAIR = 2  # L-chunks per DMA (1 MiB transfers)
GROUP = 4  # chunks per exp/matmul group
NGROUP = NCHUNK // GROUP
SCALE = 1.0 / 32.0  # 1/sqrt(H), exact power of two
MASK_BIG = 3.2e31  # (mask-1)*MASK_BIG*SCALE = -1e30 -> exp -> 0.0


# --------------------------------------------------------------------------
# Compatibility shim: the walrus build in this container accepts at most one
# sync wait and one sync update per (non-DMA) instruction, while Tile emits
# merged multi-wait sync_info. Split the extras into standalone
# EventSemaphore instructions on the same engine (FIFO order preserves
# semantics exactly).
# --------------------------------------------------------------------------

_DMA_OPCODES = {
    "DMACopy",
    "DMATranspose",
    "DMAGather",
    "DMABarrier",
    "CollectiveCompute",
    "DMATrigger",
}


def _split_sync_bir(bir: dict) -> dict:
    for f in bir.get("functions", []):
        for blk in f.get("blocks", []):
            instrs = blk.get("instructions", [])
            out = []
            for ins in instrs:
                si = ins.get("sync_info")
                if not si:
                    out.append(ins)
                    continue
                waits = si.get("on_wait") or []
                ups = si.get("on_update") or []
                pre = []
                post = []
                if len(waits) > 1:
                    for i, w in enumerate(waits[:-1]):
                        pre.append(
                            {
                                "debug": ins.get("debug", 0),
                                "engine": ins["engine"],
                                "ins": [],
                                "outs": [],
                                "name": f"{ins['name']}-sw{i}",
                                "opcode": "EventSemaphore",
                                "sync_info": {"on_update": [], "on_wait": [w]},
                            }
                        )
                    si["on_wait"] = waits[-1:]
                if len(ups) > 1 and ins.get("opcode") not in _DMA_OPCODES:
                    for i, u in enumerate(ups[1:]):
                        post.append(
                            {
                                "debug": ins.get("debug", 0),
                                "engine": ins["engine"],
                                "ins": [],
                                "outs": [],
                                "name": f"{ins['name']}-su{i}",
                                "opcode": "EventSemaphore",
                                "sync_info": {"on_update": [u], "on_wait": []},
                            }
                        )
                    si["on_update"] = ups[:1]
                out.extend(pre)
                out.append(ins)
                out.extend(post)
            blk["instructions"] = out
    return bir


def _install_compat():
    import concourse.bass2jax as b2j
    import concourse.bass_utils as bu

    if getattr(bu, "_ant_split_sync_installed", False):
        return
    orig = bu.compile_bir_kernel

    def wrapped(bir_json, tmpdir, neff_name="kernel.neff", **kw):
        bir = json.loads(bir_json)
        _split_sync_bir(bir)
        return orig(json.dumps(bir).encode(), tmpdir, neff_name=neff_name, **kw)

    bu.compile_bir_kernel = wrapped
    bu._ant_split_sync_installed = True
    if getattr(b2j, "compile_bir_kernel", None) is orig:
        b2j.compile_bir_kernel = wrapped


# --------------------------------------------------------------------------
# Kernel build
# --------------------------------------------------------------------------


def build_kernel(use_mask: bool, repeat: int = 1):
    from contextlib import ExitStack

    import concourse.bass as bass
    import concourse.tile as tile
    from concourse import mybir

    f32 = mybir.dt.float32
    i32 = mybir.dt.int32
    AF = mybir.ActivationFunctionType

    nc = bass.Bass()
    h = nc.declare_dram_parameter("h", [B_LOCAL, L, H], f32, isOutput=False)
    query = nc.declare_dram_parameter("query", [H], f32, isOutput=False)
    if use_mask:
        am = nc.declare_dram_parameter(
            "attention_mask", [B_LOCAL, L], i32, isOutput=False
        )
    out_d = nc.declare_dram_parameter("out", [B_LOCAL, H], f32, isOutput=True)

    with tile.TileContext(nc) as tc, ExitStack() as ctx:
        singles = ctx.enter_context(tc.tile_pool(name="singles", bufs=1))
        hpool = ctx.enter_context(tc.tile_pool(name="hpool", bufs=6))
        ppool = ctx.enter_context(tc.tile_pool(name="ppool", bufs=3))
        dpool = ctx.enter_context(tc.tile_pool(name="dpool", bufs=4))
        wpool = ctx.enter_context(tc.tile_pool(name="wpool", bufs=4))
        spool = ctx.enter_context(tc.tile_pool(name="spool", bufs=2))
        opool = ctx.enter_context(tc.tile_pool(name="opool", bufs=2))
        psum = ctx.enter_context(tc.tile_pool(name="psum", bufs=2, space="PSUM"))

        # Broadcast query to all 128 partitions (twice along free dim, to
        # match the [P, PAIR, H] h tiles) once at startup.
        q_b2 = singles.tile([P, PAIR, H], f32)
        q_full = query[:]
        q_bcast_ap = bass.AP(
            tensor=q_full.tensor,
            offset=q_full.offset,
            ap=[[0, P], [0, PAIR]] + list(q_full.ap),
        )
        nc.gpsimd.dma_start(out=q_b2, in_=q_bcast_ap)

        ones_col = singles.tile([P, 1], f32)
        nc.vector.memset(ones_col, 1.0)

        for b in [bb for _ in range(repeat) for bb in range(B_LOCAL)]:
            zparts = spool.tile([P, NGROUP], f32, tag="zparts")
            if use_mask:
                mask_i = spool.tile([P, NCHUNK], i32, tag="mask_i")
                nc.sync.dma_start(
                    out=mask_i, in_=am[b].rearrange("(c p) -> p c", p=P)
                )
                mask_f = spool.tile([P, NCHUNK], f32, tag="mask_f")
                nc.vector.tensor_copy(out=mask_f, in_=mask_i)
                mterm = spool.tile([P, NCHUNK], f32, tag="mterm")
                nc.vector.tensor_scalar(
                    out=mterm,
                    in0=mask_f,
                    scalar1=MASK_BIG,
                    scalar2=-MASK_BIG,
                    op0=mybir.AluOpType.mult,
                    op1=mybir.AluOpType.add,
                )

            u_ps = psum.tile([1, H], f32, tag="u")

            for g in range(NGROUP):
                dots = dpool.tile([P, GROUP], f32, tag="dots")
                hts = []
                for sub in range(GROUP // PAIR):
                    pair = g * (GROUP // PAIR) + sub
                    ht = hpool.tile([P, PAIR, H], f32, tag="ht")
                    nc.sync.dma_start(
                        out=ht,
                        in_=h[b, pair * PAIR * P : (pair + 1) * PAIR * P, :].rearrange(
                            "(n p) m -> p n m", p=P
                        ),
                    )
                    pr = ppool.tile([P, PAIR, H], f32, tag="pr")
                    nc.vector.tensor_mul(out=pr, in0=ht, in1=q_b2)
                    for n in range(PAIR):
                        k = sub * PAIR + n
                        nc.scalar.activation(
                            out=pr[:, n, :],
                            in_=pr[:, n, :],
                            func=AF.Copy,
                            accum_out=dots[:, k : k + 1],
                        )
                    hts.append(ht)

                # exp((dots + mask) / sqrt(H)); Z-partials via accum_out
                wt = wpool.tile([P, GROUP], f32, tag="wt")
                if use_mask:
                    dm = dpool.tile([P, GROUP], f32, tag="dm")
                    nc.vector.tensor_add(
                        out=dm,
                        in0=dots,
                        in1=mterm[:, g * GROUP : (g + 1) * GROUP],
                    )
                    exp_src = dm
                else:
                    exp_src = dots
                nc.scalar.activation(
                    out=wt,
                    in_=exp_src,
                    func=AF.Exp,
                    scale=SCALE,
                    accum_out=zparts[:, g : g + 1],
                )

                # PE: accumulate weighted sum of h rows into PSUM [1, H]
                for k in range(GROUP):
                    c = g * GROUP + k
                    ht = hts[k // PAIR]
                    n = k % PAIR
                    nc.tensor.matmul(
                        u_ps[:, 0:512],
                        lhsT=wt[:, k : k + 1],
                        rhs=ht[:, n, 0:512],
                        start=(c == 0),
                        stop=(c == NCHUNK - 1),
                    )
                    nc.tensor.matmul(
                        u_ps[:, 512:1024],
                        lhsT=wt[:, k : k + 1],
                        rhs=ht[:, n, 512:1024],
                        start=(c == 0),
                        stop=(c == NCHUNK - 1),
                    )

            # Z = sum over partitions and groups; out_row = U / Z
            zsum = spool.tile([P, 1], f32, tag="zsum")
            nc.vector.tensor_reduce(
                out=zsum,
                in_=zparts,
                axis=mybir.AxisListType.X,
                op=mybir.AluOpType.add,
            )
            z_ps = psum.tile([1, 1], f32, tag="z")
            nc.tensor.matmul(z_ps, lhsT=ones_col, rhs=zsum, start=True, stop=True)
            zinv = spool.tile([1, 1], f32, tag="zinv")
            nc.vector.reciprocal(out=zinv, in_=z_ps)
            out_sb = opool.tile([1, H], f32, tag="osb")
            nc.scalar.activation(
                out=out_sb, in_=u_ps, func=AF.Copy, scale=zinv
            )
            nc.sync.dma_start(out=out_d[b], in_=out_sb)

    return nc


# --------------------------------------------------------------------------
# Entry point
# --------------------------------------------------------------------------


def kernel(h, attention_mask, query):
    h = np.ascontiguousarray(np.asarray(h, dtype=np.float32))
    mask = np.asarray(attention_mask)
    q = np.ascontiguousarray(np.asarray(query, dtype=np.float32))
    assert h.shape == (B, L, H) and q.shape == (H,)

    use_mask = not bool((mask == 1).all())

    _install_compat()
    nc = build_kernel(use_mask)

    from concourse.bass_utils import run_bass_kernel_spmd

    in_maps = []
    for k in range(N_CORES):
        m = {"h": h[k * B_LOCAL : (k + 1) * B_LOCAL], "query": q}
        if use_mask:
            m["attention_mask"] = np.ascontiguousarray(
                mask[k * B_LOCAL : (k + 1) * B_LOCAL].astype(np.int32)
            )
        in_maps.append(m)

    res = run_bass_kernel_spmd(nc, in_maps, list(range(N_CORES)))
    out = np.concatenate(
        [res.results[k]["out"] for k in range(N_CORES)], axis=0
    )
    return np.asarray(out, dtype=np.float32)


if __name__ == "__main__":
    rng = np.random.default_rng(0)
    h = rng.standard_normal((B, L, H), dtype=np.float32)
    mask = np.ones((B, L), dtype=np.int32)
    q = (rng.standard_normal(H) * 0.02).astype(np.float32)
    out = kernel(h, mask, q)
    print("out", out.shape, out.dtype, out[0, :4])
